# revision 2
# baseline (speedup 1.0000x reference)
"""Blockwise butterfly rotation (nn_BlockwiseButterflyRotation) - TRN2 Bass kernel, v3.

Math: out = x @ blockdiag(C_b) with C_b the composed 256x256 butterfly
rotation of block b (8 stages), built on-device from the angles.

Layout/sharding: x.reshape(16384, 4096) row-sharded over 8 cores; each
shard is stored feature-major fp16 on the host so the device streams it
directly as the matmul moving operand with C chunks [128,128] fp16 as the
stationary operand (fast-weight-load eligible, no PE transposes):

  outT[256b + 128mc + j, r] = sum_k C_b[k, 128mc + j] * xT[256b + k, r]

v3 over v2:
- fp16 C build end to end (DVE 2x modes where APs allow).
- Butterfly stage = 4 DVE ops (t1 = state*cos_bcast, t2 = state*sin_signed
  contiguous, two half adds) with coefficients materialized by ScalarE Sin.
- HT u-replication: 16 fp16 selector matmuls; psum drained by ScalarE into
  fp16; the CT combine runs on GPSIMD (otherwise-idle engine).
- Software-pipelined passes: the C build for pass k+1 is emitted interleaved
  with the main loop of pass k (double-buffered CT/LS/HSB/angsb/coeffs), so
  in the repeat-timing harness the build costs no PE time in steady state.
- PSUM as 4x [128,1024] ring so psum drains (DVE/ACT alternating) decouple
  from PE fills.
"""
import math
import os

import numpy as np

from concourse import bacc, mybir, tile
from concourse.bass_utils import run_bass_kernel_spmd

F32 = mybir.dt.float32
F16 = mybir.dt.float16

DIM = 4096
NB = 16
BLOCK = 256
HALF_PI = math.pi / 2.0

N_CORES = 8
R_TOTAL = 4 * 4096
R_CORE = R_TOTAL // N_CORES  # 2048

# f16 consts tensor column layout: LSinit | HSBinit | W_all
_C_LS = 0          # [128, 512] LS init: delta(v == p mod 16), free (b, kc, v)
_C_HSB = 512       # [128, 512] HSB init: delta(w == 8kc + p//16), free (kc, v, w)
_C_W = 1024        # [128, 2048] W_all: free (b, mg, mu), delta(p == 16 mg + b)
_C_COLS = 3072

LAST_RESULT = None
_NC_CACHE = {}

COMBINE_ENGINE = os.environ.get("BFK_COMBINE", "gpsimd")  # gpsimd | vector


def _build_consts16() -> np.ndarray:
    c = np.zeros((128, _C_COLS), dtype=np.float16)
    p = np.arange(128)
    ls = np.zeros((128, 16, 2, 16), np.float16)
    ls[p, :, :, p % 16] = 1.0
    c[:, _C_LS:_C_LS + 512] = ls.reshape(128, 512)
    hsb = np.zeros((128, 2, 16, 16), np.float16)
    for kc in range(2):
        hsb[:, kc, :, :] = (np.arange(16)[None, :] == (8 * kc + p // 16)[:, None])[:, None, :]
    c[:, _C_HSB:_C_HSB + 512] = hsb.reshape(128, 512)
    w = np.zeros((128, 16, 8, 16), np.float16)
    for b in range(16):
        for mg in range(8):
            w[16 * mg + b, b, mg, :] = 1.0
    c[:, _C_W:_C_W + 2048] = w.reshape(128, 2048)
    return c


_CONSTS16 = _build_consts16()
_HALFPI = np.full((128, 1), HALF_PI, dtype=np.float32)


def gather_angles(angles: np.ndarray) -> np.ndarray:
    """angles [16, 8, 128] f32 -> ang [128, 1536] f32 (angL 4x256 | angH 4x128).

    Pure gather (indexing only, no arithmetic) into the per-partition
    coefficient layouts the kernel's butterfly-stage APs iterate.
    """
    angles = np.asarray(angles)
    assert angles.shape == (NB, 8, 128)
    out = np.empty((128, 1536), dtype=np.float32)
    for s in range(4):
        sig = 1 << s
        col = np.empty((128, 256), dtype=np.float32)
        for g0 in range(8):
            row = np.empty((16, 2, 8), dtype=np.float32)
            for kc in range(2):
                g = 8 * kc + g0
                for vg in range(8 // sig):
                    for t in range(sig):
                        row[:, kc, vg * sig + t] = angles[:, s, 8 * g + vg * sig + t]
            col[16 * g0:16 * g0 + 16, :] = row.reshape(1, 256)
        out[:, 256 * s:256 * (s + 1)] = col
    for sp in range(4):
        sigp = 1 << sp
        col = np.empty((128, 128), dtype=np.float32)
        for b in range(16):
            row = np.empty((16, 8), dtype=np.float32)
            for v in range(16):
                for wg in range(8 // sigp):
                    for t in range(sigp):
                        row[v, wg * sigp + t] = angles[b, sp + 4, wg * 16 * sigp + 16 * t + v]
            col[b::16, :] = row.reshape(1, 128)
        out[:, 1024 + 128 * sp:1024 + 128 * (sp + 1)] = col
    return out


def make_in_maps(x: np.ndarray, angles: np.ndarray):
    """Full x (.., 4096) f32 + angles -> per-core input maps (row shards,
    stored feature-major fp16)."""
    xf = np.asarray(x).reshape(R_TOTAL, DIM)
    x16 = xf.astype(np.float16)
    ang = gather_angles(np.asarray(angles).astype(np.float32, copy=False))
    in_maps = []
    for c in range(N_CORES):
        sh = np.ascontiguousarray(x16[c * R_CORE:(c + 1) * R_CORE, :].T)
        in_maps.append({"x": sh, "ang": ang, "consts16": _CONSTS16,
                        "halfpi": _HALFPI})
    return in_maps


class _Build:
    """Emission helpers for one C-build (into one CT ring slot)."""

    def __init__(self, nc, tc, bpool, consts16, halfpi, ANG, psR):
        self.nc, self.tc = nc, tc
        self.bpool = bpool
        self.consts16 = consts16
        self.halfpi = halfpi
        self.ANG = ANG
        self.psR = psR

    def emit_front(self, CT):
        """angles DMA, sin/cos coeffs, butterfly stages (ACT + DVE work).
        Returns state needed by emit_block."""
        nc, bpool = self.nc, self.bpool
        angsb = bpool.tile([128, 1536], F32, name="angsb", tag="angsb", bufs=2)
        nc.sync.dma_start(out=angsb[:], in_=self.ANG)

        LS = bpool.tile([128, 512], F16, name="LS", tag="LS", bufs=2)
        nc.vector.tensor_copy(LS[:], self.consts16[:, _C_LS:_C_LS + 512])
        HSB = bpool.tile([128, 512], F16, name="HSB", tag="HSB", bufs=2)
        nc.vector.tensor_copy(HSB[:], self.consts16[:, _C_HSB:_C_HSB + 512])

        Sin = mybir.ActivationFunctionType.Sin
        # HSB stages 4-7 first (they gate the selector matmuls).
        # state HSB: [p, (kc, v, vg, h, t)]; coeffs independent of kc ->
        # apply per kc-slice (contiguous [128, 256]).
        for sp in range(4):
            sigp = 1 << sp
            ng = 8 // sigp
            asl = angsb[:, 1024 + 128 * sp:1024 + 128 * (sp + 1)]
            asl3 = asl.rearrange("p (v vg t) -> p v vg t", v=16, vg=ng, t=sigp)
            cosH = bpool.tile([128, 256], F16, name="cosH", tag=f"cosH{sp}", bufs=2)
            sinH = bpool.tile([128, 256], F16, name="sinH", tag=f"sinH{sp}", bufs=2)
            cv = cosH[:].rearrange("p (v vg h t) -> p v vg h t",
                                   v=16, vg=ng, h=2, t=sigp)
            sv = sinH[:].rearrange("p (v vg h t) -> p v vg h t",
                                   v=16, vg=ng, h=2, t=sigp)
            nc.scalar.activation(cv[:, :, :, 0, :], asl3, Sin,
                                 bias=self.halfpi, scale=1.0)
            nc.scalar.activation(cv[:, :, :, 1, :], asl3, Sin,
                                 bias=self.halfpi, scale=1.0)
            nc.scalar.activation(sv[:, :, :, 0, :], asl3, Sin,
                                 bias=0.0, scale=1.0)
            nc.scalar.activation(sv[:, :, :, 1, :], asl3, Sin,
                                 bias=0.0, scale=-1.0)
            for kc in range(2):
                stk = HSB[:, kc * 256:(kc + 1) * 256]
                self._stage(stk, cosH[:], sinH[:],
                            lambda ap, ng=ng, sigp=sigp: ap.rearrange(
                                "p (v vg h t) -> p v vg h t",
                                v=16, vg=ng, h=2, t=sigp))

        # LS stages 0-3; state LS: [p, (b, kc, vg, h, t)] (v = (vg, h, t)).
        for s in range(4):
            sig = 1 << s
            ng = 8 // sig
            asl = angsb[:, 256 * s:256 * (s + 1)]
            asl4 = asl.rearrange("p (b kc vg t) -> p b kc vg t",
                                 b=16, kc=2, vg=ng, t=sig)
            cosL = bpool.tile([128, 512], F16, name="cosL", tag=f"cosL{s}", bufs=2)
            sinL = bpool.tile([128, 512], F16, name="sinL", tag=f"sinL{s}", bufs=2)
            cv = cosL[:].rearrange("p (b kc vg h t) -> p b kc vg h t",
                                   b=16, kc=2, vg=ng, h=2, t=sig)
            sv = sinL[:].rearrange("p (b kc vg h t) -> p b kc vg h t",
                                   b=16, kc=2, vg=ng, h=2, t=sig)
            nc.scalar.activation(cv[:, :, :, :, 0, :], asl4, Sin,
                                 bias=self.halfpi, scale=1.0)
            nc.scalar.activation(cv[:, :, :, :, 1, :], asl4, Sin,
                                 bias=self.halfpi, scale=1.0)
            nc.scalar.activation(sv[:, :, :, :, 0, :], asl4, Sin,
                                 bias=0.0, scale=1.0)
            nc.scalar.activation(sv[:, :, :, :, 1, :], asl4, Sin,
                                 bias=0.0, scale=-1.0)
            self._stage(LS[:], cosL[:], sinL[:],
                        lambda ap, ng=ng, sig=sig: ap.rearrange(
                            "p (b kc vg h t) -> p b kc vg h t",
                            b=16, kc=2, vg=ng, h=2, t=sig))

        return LS, HSB

    def _stage(self, st, cosF, sinF, view):
        """One butterfly stage on contiguous state slice `st` [128, n].
        cosF: h-duplicated cos, sinF: h-signed sin (h=0: +s, h=1: -s),
        both contiguous with the same layout. view() lifts a flat AP to the
        [..., h, t] shape. state = state*c (+/-) swap_h(state)*s."""
        nc, bpool = self.nc, self.bpool
        n = st.free_size()
        t1 = bpool.tile([128, n], F16, name="bt1", tag=f"bt1_{n}", bufs=2)
        t2 = bpool.tile([128, n], F16, name="bt2", tag=f"bt2_{n}", bufs=2)
        nc.vector.tensor_mul(t1[:], st, cosF)
        nc.vector.tensor_mul(t2[:], st, sinF)
        stv = view(st)
        t1v = view(t1[:])
        t2v = view(t2[:])
        nc.vector.tensor_add(stv[..., 0, :], t1v[..., 0, :], t2v[..., 1, :])
        nc.vector.tensor_add(stv[..., 1, :], t1v[..., 1, :], t2v[..., 0, :])

    def emit_block(self, CT, LS, HSB, b):
        """Selector matmul + hss drain + combine for one block b."""
        nc = self.nc
        Wb = self.consts16[:, _C_W + 128 * b:_C_W + 128 * (b + 1)]
        psr = self.psR.tile([128, 512], F32, name="psr", tag="psr", bufs=2)
        nc.tensor.matmul(psr[:], Wb, HSB[:], start=True, stop=True)
        hss = self.bpool.tile([128, 512], F16, name="hss", tag="hss", bufs=2)
        nc.scalar.copy(hss[:], psr[:])
        o = CT[:, b * 512:(b + 1) * 512] \
            .rearrange("p (kc w v) -> p kc w v", kc=2, w=16, v=16)
        i0 = LS[:, b * 32:(b + 1) * 32] \
            .rearrange("p (kc v) -> p kc v", kc=2, v=16) \
            .unsqueeze(2).to_broadcast((128, 2, 16, 16))
        i1 = hss[:].rearrange("p (kc v w) -> p kc w v", kc=2, v=16, w=16)
        if COMBINE_ENGINE == "gpsimd":
            nc.gpsimd.tensor_mul(o, i0, i1)
        else:
            nc.vector.tensor_mul(o, i0, i1)


def build_nc(R: int, repeat: int | None = None, repeat_scope: str = "all",
             debug_counter: bool = False):
    """repeat: wrap the body in an on-device For_i re-running it `repeat`
    times (must be even; two software-pipelined passes per loop body)."""
    assert R % 512 == 0
    NRG = R // 512
    nc = bacc.Bacc("TRN2", target_bir_lowering=False, debug=False)

    X = nc.dram_tensor("x", [DIM, R], F16, kind="ExternalInput").ap()
    ANG = nc.dram_tensor("ang", [128, 1536], F32, kind="ExternalInput").ap()
    CIN16 = nc.dram_tensor("consts16", [128, _C_COLS], F16, kind="ExternalInput").ap()
    HPI = nc.dram_tensor("halfpi", [128, 1], F32, kind="ExternalInput").ap()
    OUT = nc.dram_tensor("out", [DIM, R], F16, kind="ExternalOutput").ap()
    CNT = (nc.dram_tensor("cnt", [128, 8], F32, kind="ExternalOutput").ap()
           if debug_counter else None)

    with tile.TileContext(nc) as tc:
        with tc.tile_pool(name="const", bufs=1) as cpool, \
             tc.tile_pool(name="ctp", bufs=1) as ctpool, \
             tc.tile_pool(name="build", bufs=1) as bpool, \
             tc.tile_pool(name="xin", bufs=3) as xpool, \
             tc.tile_pool(name="outp", bufs=4) as opool, \
             tc.tile_pool(name="psO", bufs=3, space="PSUM") as psO, \
             tc.tile_pool(name="psR", bufs=1, space="PSUM") as psR:
            consts16 = cpool.tile([128, _C_COLS], F16)
            nc.sync.dma_start(out=consts16[:], in_=CIN16)
            halfpi_t = cpool.tile([128, 1], F32)
            nc.sync.dma_start(out=halfpi_t[:], in_=HPI)
            halfpi = halfpi_t[:, 0:1]

            bld = _Build(nc, tc, bpool, consts16[:], halfpi, ANG, psR)

            def new_ct():
                return ctpool.tile([128, 8192], F16, name="CT", tag="CT", bufs=2)

            def emit_build_serial(CT):
                LS, HSB = bld.emit_front(CT)
                for b in range(16):
                    bld.emit_block(CT, LS, HSB, b)

            def emit_main_mc(CT, xb, b, half, dve_first):
                mc = 2 * b + half
                t1 = psO.tile([128, 1024], F32, name="ps", tag="ps")
                t2 = psO.tile([128, 1024], F32, name="ps", tag="ps")
                for kc in range(2):
                    w = CT[:, (2 * b + kc) * 256 + 128 * half:
                           (2 * b + kc) * 256 + 128 * half + 128]
                    for rg in range(NRG):
                        tgt = t1 if rg < 2 else t2
                        nc.tensor.matmul(
                            tgt[:, 512 * (rg % 2):512 * (rg % 2 + 1)],
                            w,
                            xb[:, kc * R + 512 * rg:kc * R + 512 * (rg + 1)],
                            start=(kc == 0), stop=(kc == 1))
                ot = opool.tile([128, R], F16, name="ot", tag="ot")
                if dve_first:
                    nc.vector.tensor_copy(ot[:, 0:1024], t1[:])
                    nc.scalar.copy(ot[:, 1024:2048], t2[:])
                else:
                    nc.scalar.copy(ot[:, 0:1024], t1[:])
                    nc.vector.tensor_copy(ot[:, 1024:2048], t2[:])
                nc.scalar.dma_start(out=OUT[128 * mc:128 * (mc + 1), :], in_=ot[:])

            def emit_pass(CT_cur, CT_nxt):
                """One full pass: main loop reading CT_cur, interleaved with
                the build of CT_nxt (if CT_nxt is not None)."""
                state = {}
                if CT_nxt is not None:
                    state["front"] = bld.emit_front(CT_nxt)
                nblk = 0
                for b in range(16):
                    xb = xpool.tile([128, 2 * R], F16, name="xb", tag="xb")
                    nc.sync.dma_start(out=xb[:, 0:R],
                                      in_=X[256 * b:256 * b + 128, :])
                    nc.sync.dma_start(out=xb[:, R:2 * R],
                                      in_=X[256 * b + 128:256 * b + 256, :])
                    for half in range(2):
                        mc = 2 * b + half
                        emit_main_mc(CT_cur, xb, b, half, dve_first=(mc % 2 == 0))
                        if CT_nxt is not None and 6 <= mc < 22:
                            LS, HSB = state["front"]
                            bld.emit_block(CT_nxt, LS, HSB, nblk)
                            nblk += 1
                if CT_nxt is not None:
                    assert nblk == 16
                if cnt_t is not None:
                    nc.vector.tensor_scalar_add(cnt_t[:], cnt_t[:], 1.0)

            cnt_t = None
            if debug_counter:
                cnt_t = cpool.tile([128, 8], F32)
                nc.vector.memset(cnt_t[:], 0.0)
            # prologue: serial build of the first CT
            CT_a = new_ct()
            emit_build_serial(CT_a)

            if repeat is None:
                emit_pass(CT_a, None)
                if cnt_t is not None:
                    nc.gpsimd.dma_start(out=CNT, in_=cnt_t[:])
            else:
                assert repeat % 2 == 0 and repeat >= 2
                with tc.For_i(0, repeat // 2, 1):
                    CT_b = new_ct()
                    emit_pass(CT_a, CT_b)   # builds CT_b (slot 1)
                    CT_c = new_ct()         # same slot as CT_a
                    emit_pass(CT_b, CT_c)
                if cnt_t is not None:
                    nc.gpsimd.dma_start(out=CNT, in_=cnt_t[:])

    nc.compile()
    return nc


def _get_nc():
    if "nc" not in _NC_CACHE:
        _NC_CACHE["nc"] = build_nc(R_CORE)
    return _NC_CACHE["nc"]


def kernel(x: np.ndarray, angles: np.ndarray) -> np.ndarray:
    global LAST_RESULT
    x = np.asarray(x)
    orig_shape = x.shape
    in_maps = make_in_maps(x, angles)

    nc = _get_nc()
    trace = os.environ.get("BFK_TRACE", "") == "1"
    res = run_bass_kernel_spmd(nc, in_maps, list(range(N_CORES)), trace=trace)
    LAST_RESULT = res
    out = np.empty((R_TOTAL, DIM), dtype=np.float32)
    for c in range(N_CORES):
        out[c * R_CORE:(c + 1) * R_CORE, :] = res.results[c]["out"].T
    return out.reshape(orig_shape).astype(x.dtype, copy=False)
